# revision 1
# baseline (speedup 1.0000x reference)
"""DIEN (Deep Interest Evolution Network) Trainium2 Bass kernel.

Data-parallel over 8 NeuronCores, 128 batch rows per core:
  - Embedding gathers on-device via indirect DMA (batched timesteps/call).
  - Compute in "transposed" layout [feature_on_partitions, batch_on_free] so
    recurrent matmuls need no per-step transposes (weights host-pre-transposed
    into lhsT layout [K, M]).
  - GRU / AUGRU scans run as two independent 64-row chains per core,
    interleaved so engine latencies of one chain hide under the other.
  - AUGRU attention scalar a_t broadcast across partitions via a replicating
    HWDGE DMA read from a DRAM staging buffer (no compute-engine cost).
"""

import os
import sys
from contextlib import ExitStack

for _p in ("/opt/trn_rl_repo", "/root/.axon_site/_ro/trn_rl_repo"):
    if os.path.isdir(_p) and _p not in sys.path:
        sys.path.insert(0, _p)

import numpy as np

import concourse.bass as bass
import concourse.mybir as mybir
import concourse.tile as tile
from concourse import bacc
from concourse.bass import IndirectOffsetOnAxis
from concourse.bass_utils import run_bass_kernel_spmd
from concourse.masks import make_identity

F32 = mybir.dt.float32
F32R = mybir.dt.float32r
I32 = mybir.dt.int32
AF = mybir.ActivationFunctionType
OP = mybir.AluOpType
AX = mybir.AxisListType

B, D, H, V = 1024, 128, 128, 100000
NCORES = 8
BS = B // NCORES          # 128 batch rows per core
HB = BS // 2              # 64 rows per chain
P = 128
GCHUNK = 16               # timesteps per indirect-gather call


def _emit(nc, T, ap):
    """Emit the whole per-core program. `ap` maps name -> DRAM AP."""
    with tile.TileContext(nc) as tc, ExitStack() as st:
        const = st.enter_context(tc.tile_pool(name="const", bufs=1))

        # ---- constants / weights into SBUF ----
        ident = const.tile([P, P], F32)
        make_identity(nc, ident[:])


        def load(name, shape, dt=F32):
            t = const.tile(shape, dt, name=name, tag=name)
            nc.sync.dma_start(t[:], ap[name][:])
            return t

        g_wih = load("g_wihT", [P, 3 * H], F32)   # [D, r|z|n]
        g_whh = load("g_whhT", [P, 3 * H], F32)
        a_wih = load("a_wihT", [P, 3 * H], F32)
        a_whh = load("a_whhT", [P, 3 * H], F32)
        attw1 = load("attw1", [P, 192], F32)      # [D, Ai|Aq|Ap]
        attw2 = load("attw2", [64, 32], F32)
        attwo2 = load("attwo2", [32, 2], F32)
        attb1 = load("attb1", [64, 1])
        attb2 = load("attb2", [32, 1])
        mlpw1a = const.tile([P, 256], F32)
        nc.sync.dma_start(mlpw1a[:], ap["mlpw1"][0:128, :])
        mlpw1b = const.tile([P, 256], F32)
        nc.sync.dma_start(mlpw1b[:], ap["mlpw1"][128:256, :])
        mlpw2a = const.tile([P, 128], F32)
        nc.sync.dma_start(mlpw2a[:], ap["mlpw2"][0:128, :])
        mlpw2b = const.tile([P, 128], F32)
        nc.sync.dma_start(mlpw2b[:], ap["mlpw2"][128:256, :])
        mlpb1a = const.tile([P, 1], F32)
        nc.sync.dma_start(mlpb1a[:], ap["mlpb1"][0:128, :])
        mlpb1b = const.tile([P, 1], F32)
        nc.sync.dma_start(mlpb1b[:], ap["mlpb1"][128:256, :])
        mlpb2 = load("mlpb2", [128, 1])
        outw = load("outw", [P, 1], F32)
        outb = load("outb", [1, 1])
        maskb = load("maskb", [P, T])        # 0 / -1e9 additive mask, [b, t]

        gidx_sb = const.tile([P, T], I32)
        nc.sync.dma_start(gidx_sb[:], ap["gidx"][:])
        tidx_sb = const.tile([P, 1], I32)
        nc.sync.dma_start(tidx_sb[:], ap["tidx"][:])

        zeros = const.tile([P, P], F32)
        nc.vector.memset(zeros[:], 0.0)

        # big persistent buffers
        interest = const.tile([P, T * P], F32)   # [H, (t,b)] transposed
        haug = const.tile([P, P], F32)           # final AUGRU hidden [H, b]
        att_nat = const.tile([P, T], F32)        # attention weights [b, t]
        itemT = const.tile([P, P], F32)          # target item emb [D, b]
        q8 = const.tile([P, 8 * P], F32)         # itemT replicated 8x in free

        def mm(out, lhsT, rhs, start, stop):
            nc.tensor.matmul(out, lhsT, rhs, start=start, stop=stop)

        def it_slice(t, c):
            return interest[:, t * P + c * HB: t * P + (c + 1) * HB]

        def scan_step(pbank, gates, wih, whh, x_rhs, hprev, bc, out_slices, dbg=None):
            """One timestep for both 64-row chains.

            x_rhs(c) / hprev(c) / out_slices(c): APs for chain c.
            bc: [128,128] a_t broadcast tile, or None => plain GRU update.
            """
            banks = []
            for c in (0, 1):
                bank = pbank.tile([P, 256], F32, tag=f"bank{c}")
                r_, z_ = bank[:, 0:64], bank[:, 64:128]
                xn_, hn_ = bank[:, 128:192], bank[:, 192:256]
                xr = x_rhs(c)
                mm(r_, wih[:, 0:H], xr, start=True, stop=False)
                mm(z_, wih[:, H:2 * H], xr, start=False, stop=False)
                mm(xn_, wih[:, 2 * H:3 * H], xr, start=False, stop=False)
                hp = hprev(c)
                mm(hn_, whh[:, 2 * H:3 * H], hp, start=False, stop=False)
                mm(r_, whh[:, 0:H], hp, start=False, stop=False)
                mm(z_, whh[:, H:2 * H], hp, start=False, stop=True)
                banks.append(bank)
            rzs, ns, ws = [], [], []
            for c in (0, 1):
                rz = gates.tile([P, P], F32, tag=f"rz{c}")
                nc.scalar.activation(rz[:], banks[c][:, 0:128], AF.Sigmoid)
                if dbg is not None and c == 0:
                    nc.sync.dma_start(dbg["rz0"][:], rz[:])
                    xnhn = gates.tile([P, P], F32, tag="dbgxn", name="xnhn")
                    nc.vector.tensor_copy(xnhn[:], banks[c][:, 128:256])
                    nc.sync.dma_start(dbg["xnhn0"][:], xnhn[:])
                rzs.append(rz)
                u = gates.tile([P, HB], F32, tag=f"u{c}")
                nc.vector.tensor_mul(u[:], rz[:, 0:64], banks[c][:, 192:256])
                v = gates.tile([P, HB], F32, tag=f"v{c}")
                nc.vector.tensor_add(v[:], u[:], banks[c][:, 128:192])
                n = gates.tile([P, HB], F32, tag=f"n{c}")
                nc.scalar.activation(n[:], v[:], AF.Tanh)
                ns.append(n)
                w = gates.tile([P, HB], F32, tag=f"w{c}")
                nc.gpsimd.tensor_sub(w[:], hprev(c), n[:])
                ws.append(w)
            for c in (0, 1):
                outsl = out_slices(c)
                if bc is None:
                    # GRU: h' = n + z*(h-n)
                    y = gates.tile([P, HB], F32, tag=f"y{c}")
                    nc.gpsimd.tensor_mul(y[:], rzs[c][:, 64:128], ws[c][:])
                    nc.vector.tensor_add(outsl, ns[c][:], y[:])
                else:
                    # AUGRU: h' = h - a*z*(h-n)
                    q = gates.tile([P, HB], F32, tag=f"q{c}")
                    nc.vector.tensor_mul(q[:], rzs[c][:, 64:128],
                                         bc[:, c * HB:(c + 1) * HB])
                    y = gates.tile([P, HB], F32, tag=f"y{c}")
                    nc.gpsimd.tensor_mul(y[:], q[:], ws[c][:])
                    nc.vector.tensor_sub(outsl, hprev(c), y[:])

        # =========================================================
        # Phase 0+1: item embedding, then GRU over T steps
        # =========================================================
        with tc.tile_pool(name="gnat", bufs=6) as gnat, \
             tc.tile_pool(name="xpos", bufs=4) as xpos, \
             tc.tile_pool(name="pbank", bufs=2, space="PSUM") as pbank, \
             tc.tile_pool(name="ptr", bufs=2, space="PSUM") as ptr, \
             tc.tile_pool(name="gates", bufs=3) as gates:

            item_nat = gnat.tile([P, P], F32, tag="item")
            nc.gpsimd.indirect_dma_start(
                out=item_nat[:], out_offset=None, in_=ap["emb"][:],
                in_offset=IndirectOffsetOnAxis(ap=tidx_sb[:, 0:1], axis=0))
            it_ps = ptr.tile([P, P], F32, tag="tr")
            nc.tensor.transpose(it_ps[:], item_nat[:], ident[:])
            nc.scalar.copy(itemT[:], it_ps[:])
            nc.vector.tensor_copy(q8[:], itemT[:].unsqueeze(1).to_broadcast([P, 8, P]))

            for t in range(T):
                gn = gnat.tile([P, P], F32, tag="hist", name="gn")
                nc.gpsimd.indirect_dma_start(
                    out=gn[:], out_offset=None, in_=ap["emb"][:],
                    in_offset=IndirectOffsetOnAxis(
                        ap=gidx_sb[:, t:t + 1], axis=0))
                if True:
                    tp = ptr.tile([P, P], F32, tag="tr")
                    nc.tensor.transpose(tp[:], gn[:], ident[:])
                    xT = xpos.tile([P, P], F32, tag="xT")
                    if t % 2 == 0:
                        nc.scalar.copy(xT[:], tp[:])
                    else:
                        nc.vector.tensor_copy(xT[:], tp[:])
                    scan_step(
                        pbank, gates, g_wih, g_whh,
                        x_rhs=lambda c: xT[:, c * HB:(c + 1) * HB],
                        hprev=(lambda c: zeros[:, c * HB:(c + 1) * HB])
                        if t == 0 else (lambda c: it_slice(t - 1, c)),
                        bc=None,
                        out_slices=lambda c: it_slice(t, c),
                        dbg=({k: ap[k] for k in ("rz0", "xnhn0")}
                             if t == 0 and "rz0" in ap else None))

        if "dbg_interest" in ap:
            nc.sync.dma_start(ap["dbg_interest"][:], interest[:])

        # =========================================================
        # Phase 2: attention scores + softmax + transpose to DRAM
        # =========================================================
        NT = T * P
        GRP = 1024
        with tc.tile_pool(name="apsum", bufs=2, space="PSUM") as apsum, \
             tc.tile_pool(name="ap2", bufs=1, space="PSUM") as ap2, \
             tc.tile_pool(name="asc", bufs=1, space="PSUM") as asc, \
             tc.tile_pool(name="asb", bufs=3) as asb:
            scores_ps = asc.tile([P, 2 * T], F32, tag="scores")
            for gi in range((NT + GRP - 1) // GRP):
                c0 = gi * GRP
                cw = min(GRP, NT - c0)
                iq = asb.tile([P, GRP], F32, tag="iq")
                eng = nc.vector if gi % 2 == 0 else nc.gpsimd
                eng.tensor_mul(iq[:, 0:cw], interest[:, c0:c0 + cw],
                               q8[:, 0:cw])
                ps1 = apsum.tile([64, GRP], F32, tag="ps1")
                for h2 in range(0, cw, 512):
                    hw = min(512, cw - h2)
                    seg = ps1[:, h2:h2 + hw]
                    mm(seg, attw1[:, 0:64],
                       interest[:, c0 + h2:c0 + h2 + hw],
                       start=True, stop=False)
                    mm(seg, attw1[:, 64:128], q8[:, 0:hw], start=False, stop=False)
                    mm(seg, attw1[:, 128:192], iq[:, h2:h2 + hw], start=False, stop=True)
                r1 = asb.tile([64, GRP], F32, tag="r1")
                nc.scalar.activation(r1[:, 0:cw], ps1[:, 0:cw], AF.Relu,
                                     bias=attb1[:])
                ps2 = ap2.tile([32, GRP], F32, tag="ps2")
                for h2 in range(0, cw, 512):
                    hw = min(512, cw - h2)
                    mm(ps2[:, h2:h2 + hw], attw2[:], r1[:, h2:h2 + hw], start=True, stop=True)
                r2 = asb.tile([32, GRP], F32, tag="r2")
                nc.scalar.activation(r2[:, 0:cw], ps2[:, 0:cw], AF.Relu,
                                     bias=attb2[:])
                for j in range(cw // P):
                    t = (c0 + j * P) // P
                    mm(scores_ps[:, 2 * t:2 * t + 2],
                       r2[:, j * P:(j + 1) * P], attwo2[:],
                       start=True, stop=True)

            sm = asb.tile([P, T], F32, tag="sm")
            nc.vector.tensor_add(sm[:], scores_ps[:, 0:2 * T:2], maskb[:])
            mx = asb.tile([P, 1], F32, tag="mx")
            nc.vector.tensor_reduce(mx[:], sm[:], axis=AX.X, op=OP.max)
            nmx = asb.tile([P, 1], F32, tag="nmx")
            nc.vector.tensor_scalar_mul(nmx[:], mx[:], -1.0)
            ex = asb.tile([P, T], F32, tag="ex")
            nc.scalar.activation(ex[:], sm[:], AF.Exp, bias=nmx[:])
            sume = asb.tile([P, 1], F32, tag="sume")
            nc.vector.tensor_reduce(sume[:], ex[:], axis=AX.X, op=OP.add)
            rec = asb.tile([P, 1], F32, tag="rec")
            nc.vector.reciprocal(rec[:], sume[:])
            nc.vector.tensor_scalar_mul(att_nat[:], ex[:], rec[:])

        with tc.tile_pool(name="atr", bufs=2, space="PSUM") as atr, \
             tc.tile_pool(name="atsb", bufs=2) as atsb:
            for k in range((T + P - 1) // P):
                tw = min(P, T - k * P)
                atp = atr.tile([P, P], F32, tag="tr")
                nc.tensor.transpose(atp[0:tw, :],
                                    att_nat[:, k * P:k * P + tw], ident[:])
                ats = atsb.tile([P, P], F32, tag="ats")
                nc.scalar.copy(ats[0:tw, :], atp[0:tw, :])
                nc.sync.dma_start(ap["attTd"][k * P:k * P + tw, :],
                                  ats[0:tw, :])

        if "dbg_att" in ap:
            nc.sync.dma_start(ap["dbg_att"][:], att_nat[:])

        # =========================================================
        # Phase 3: AUGRU
        # =========================================================
        with tc.tile_pool(name="pbank2", bufs=2, space="PSUM") as pbank, \
             tc.tile_pool(name="gates2", bufs=3) as gates, \
             tc.tile_pool(name="hpool", bufs=4) as hpool, \
             tc.tile_pool(name="bcp", bufs=4) as bcp:
            ah_prev = [None, None]
            for t in range(T):
                bc = bcp.tile([P, P], F32, tag="bc")
                nc.sync.dma_start(
                    bc[:], ap["attTd"][t:t + 1, :].to_broadcast([P, P]))
                ah_new = [hpool.tile([P, HB], F32, tag=f"ah{c}",
                                     name=f"ah{c}")
                          for c in (0, 1)] if t < T - 1 else None

                def out_sl(c, t=t, ah_new=ah_new):
                    if t == T - 1:
                        return haug[:, c * HB:(c + 1) * HB]
                    return ah_new[c][:]

                scan_step(
                    pbank, gates, a_wih, a_whh,
                    x_rhs=lambda c, t=t: it_slice(t, c),
                    hprev=(lambda c: zeros[:, c * HB:(c + 1) * HB]) if t == 0
                    else (lambda c, hp=ah_prev: hp[c][:]),
                    bc=bc,
                    out_slices=out_sl)
                ah_prev = ah_new

        # =========================================================
        # Phase 4: final MLP
        # =========================================================
        with tc.tile_pool(name="mpsum", bufs=1, space="PSUM") as mpsum, \
             tc.tile_pool(name="msb", bufs=1) as msb:
            hdn = []
            for m in range(2):
                ps = mpsum.tile([P, P], F32, tag=f"mlp1_{m}")
                mm(ps[:], mlpw1a[:, m * 128:(m + 1) * 128], haug[:], start=True, stop=False)
                mm(ps[:], mlpw1b[:, m * 128:(m + 1) * 128], itemT[:], start=False, stop=True)
                hd = msb.tile([P, P], F32, tag=f"hdn{m}")
                nc.scalar.activation(hd[:], ps[:], AF.Relu,
                                     bias=(mlpb1a if m == 0 else mlpb1b)[:])
                hdn.append(hd)
            ps = mpsum.tile([P, P], F32, tag="mlp2")
            mm(ps[:], mlpw2a[:], hdn[0][:], start=True, stop=False)
            mm(ps[:], mlpw2b[:], hdn[1][:], start=False, stop=True)
            hd2 = msb.tile([P, P], F32, tag="hdn2")
            nc.scalar.activation(hd2[:], ps[:], AF.Relu, bias=mlpb2[:])
            pso = mpsum.tile([1, P], F32, tag="mlpo")
            mm(pso[:], outw[:], hd2[:], start=True, stop=True)
            res = msb.tile([1, P], F32, tag="res")
            nc.scalar.activation(res[:], pso[:], AF.Identity, bias=outb[:])
            nc.sync.dma_start(ap["out"][:], res[:])


def _build(T):
    nc = bacc.Bacc("TRN2", target_bir_lowering=False, debug=False)
    ap = {}

    def din(name, shape, dt=F32):
        ap[name] = nc.dram_tensor(name, shape, dt, kind="ExternalInput").ap()

    din("emb", [V, D], F32)
    din("gidx", [BS, T], I32)
    din("tidx", [BS, 1], I32)
    din("maskb", [BS, T])
    din("g_wihT", [D, 3 * H], F32); din("g_whhT", [H, 3 * H], F32)
    din("a_wihT", [H, 3 * H], F32); din("a_whhT", [H, 3 * H], F32)
    din("attw1", [D, 192], F32); din("attw2", [64, 32], F32); din("attwo2", [32, 2], F32)
    din("attb1", [64, 1]); din("attb2", [32, 1])
    din("mlpw1", [256, 256], F32); din("mlpb1", [256, 1])
    din("mlpw2", [256, 128], F32); din("mlpb2", [128, 1])
    din("outw", [128, 1], F32); din("outb", [1, 1])
    ap["attTd"] = nc.dram_tensor("attTd", [((T + P - 1) // P) * P, P], F32).ap()
    if os.environ.get("DIEN_DEBUG"):
        ap["dbg_interest"] = nc.dram_tensor(
            "dbg_interest", [P, T * P], F32, kind="ExternalOutput").ap()
        ap["dbg_att"] = nc.dram_tensor(
            "dbg_att", [P, T], F32, kind="ExternalOutput").ap()
        ap["rz0"] = nc.dram_tensor(
            "rz0", [P, P], F32, kind="ExternalOutput").ap()
        ap["xnhn0"] = nc.dram_tensor(
            "xnhn0", [P, P], F32, kind="ExternalOutput").ap()
    ap["out"] = nc.dram_tensor("out", [1, BS], F32, kind="ExternalOutput").ap()
    _emit(nc, T, ap)
    nc.compile()
    return nc


def _prep_inputs(inputs, T):
    """Host-side sharding / weight layout prep. Returns list of 8 dicts."""
    f = lambda x: np.ascontiguousarray(np.asarray(x, dtype=np.float32))
    emb = f(inputs["emb"])
    hist = np.asarray(inputs["history_items"]).astype(np.int32)
    tgt = np.asarray(inputs["target_item"]).astype(np.int32)
    mask = np.asarray(inputs["history_mask"])
    maskb = np.where(mask, 0.0, -1e9).astype(np.float32)

    def gate_T(w):  # [3H, K] -> [K, 3H] with r|z|n column blocks
        return np.ascontiguousarray(f(w).T)

    for nm in ("gru_bih", "gru_bhh", "augru_bih", "augru_bhh"):
        assert np.abs(np.asarray(inputs[nm])).max() == 0.0, \
            f"nonzero bias {nm} unsupported"
    W1 = f(inputs["att_W1"])  # [4H, 64]
    Ai = W1[0:H] + W1[2 * H:3 * H]
    Aq = W1[H:2 * H] - W1[2 * H:3 * H]
    Apr = W1[3 * H:4 * H]
    attw1 = np.ascontiguousarray(np.concatenate([Ai, Aq, Apr], axis=1))
    common = dict(
        emb=emb,
        g_wihT=gate_T(inputs["gru_Wih"]), g_whhT=gate_T(inputs["gru_Whh"]),
        a_wihT=gate_T(inputs["augru_Wih"]), a_whhT=gate_T(inputs["augru_Whh"]),
        attw1=attw1, attw2=f(inputs["att_W2"]), attwo2=np.ascontiguousarray(np.repeat(f(inputs["att_Wo"]), 2, axis=1)),
        attb1=f(inputs["att_b1"]).reshape(64, 1),
        attb2=f(inputs["att_b2"]).reshape(32, 1),
        mlpw1=f(inputs["mlp_W1"]), mlpb1=f(inputs["mlp_b1"]).reshape(256, 1),
        mlpw2=f(inputs["mlp_W2"]), mlpb2=f(inputs["mlp_b2"]).reshape(128, 1),
        outw=f(inputs["out_W"]), outb=f(inputs["out_b"]).reshape(1, 1),
    )
    ins = []
    for c in range(NCORES):
        s = slice(c * BS, (c + 1) * BS)
        m = dict(common)
        m["gidx"] = np.ascontiguousarray(hist[s, :T])
        m["tidx"] = np.ascontiguousarray(tgt[s].reshape(BS, 1))
        m["maskb"] = np.ascontiguousarray(maskb[s, :T])
        ins.append(m)
    return ins


_CACHE = {}


def run(inputs, T):
    if T not in _CACHE:
        _CACHE[T] = _build(T)
    nc = _CACHE[T]
    ins = _prep_inputs(inputs, T)
    res = run_bass_kernel_spmd(nc, ins, core_ids=list(range(NCORES)))
    out = np.concatenate([res.results[c]["out"].reshape(BS)
                          for c in range(NCORES)])
    return out.reshape(B, 1).astype(np.float32)


def kernel(**inputs):
    T = np.asarray(inputs["history_items"]).shape[1]
    return run(inputs, T)



# revision 5
# speedup vs baseline: 1.0012x; 1.0012x over previous
"""DIEN (Deep Interest Evolution Network) Trainium2 Bass kernel.

Data-parallel over 8 NeuronCores, 128 batch rows per core:
  - Embedding gathers on-device via indirect DMA from an fp16 copy of the
    table (half the HBM traffic of fp32).
  - Compute in "transposed" layout [feature_on_partitions, batch_on_free] so
    recurrent matmuls need no per-step transposes (weights host-pre-transposed
    into lhsT layout [K, M]).
  - All matmul operands are fp16 (weights + activations); PSUM accumulates in
    fp32.  fp16 matmuls stream 1 cycle/row on the PE vs 4 for fp32.
  - GRU / AUGRU scans run as two independent 64-row chains per core,
    interleaved so engine latencies of one chain hide under the other.
  - AUGRU attention scalar a_t broadcast across partitions via a replicating
    HWDGE DMA read from a DRAM staging buffer (no compute-engine cost).
"""

import os
import sys
from contextlib import ExitStack

for _p in ("/opt/trn_rl_repo", "/root/.axon_site/_ro/trn_rl_repo"):
    if os.path.isdir(_p) and _p not in sys.path:
        sys.path.insert(0, _p)

import numpy as np

import concourse.bass as bass
import concourse.mybir as mybir
import concourse.tile as tile
from concourse import bacc
from concourse.bass import IndirectOffsetOnAxis
from concourse.bass_utils import run_bass_kernel_spmd
from concourse.masks import make_identity

F32 = mybir.dt.float32
F16 = mybir.dt.float16
I32 = mybir.dt.int32
AF = mybir.ActivationFunctionType
OP = mybir.AluOpType
AX = mybir.AxisListType

B, D, H, V = 1024, 128, 128, 100000
NCORES = 8
BS = B // NCORES          # 128 batch rows per core
HB = BS // 2              # 64 rows per chain
P = 128
GCHUNK = 1               # timesteps per indirect-gather call


def _emit(nc, T, ap):
    """Emit the whole per-core program. `ap` maps name -> DRAM AP."""
    with tile.TileContext(nc) as tc, ExitStack() as st:
        const = st.enter_context(tc.tile_pool(name="const", bufs=1))

        # ---- constants / weights into SBUF ----
        ident = const.tile([P, P], F16)
        make_identity(nc, ident[:])

        def load(name, shape, dt=F32):
            t = const.tile(shape, dt, name=name, tag=name)
            nc.sync.dma_start(t[:], ap[name][:])
            return t

        g_wih = load("g_wihT", [P, 3 * H], F16)   # [D, r|z|n]
        g_whh = load("g_whhT", [P, 3 * H], F16)
        a_wih = load("a_wihT", [P, 3 * H], F16)
        a_whh = load("a_whhT", [P, 3 * H], F16)
        attw1 = load("attw1", [P, 192], F16)      # [D, Ai|Aq|Ap]
        attw2 = load("attw2", [64, 32], F16)
        attwo2 = load("attwo2", [32, 2], F16)
        attb1 = load("attb1", [64, 1])
        attb2 = load("attb2", [32, 1])
        mlpw1a = const.tile([P, 256], F16)
        nc.sync.dma_start(mlpw1a[:], ap["mlpw1"][0:128, :])
        mlpw1b = const.tile([P, 256], F16)
        nc.sync.dma_start(mlpw1b[:], ap["mlpw1"][128:256, :])
        mlpw2a = const.tile([P, 128], F16)
        nc.sync.dma_start(mlpw2a[:], ap["mlpw2"][0:128, :])
        mlpw2b = const.tile([P, 128], F16)
        nc.sync.dma_start(mlpw2b[:], ap["mlpw2"][128:256, :])
        mlpb1a = const.tile([P, 1], F32)
        nc.sync.dma_start(mlpb1a[:], ap["mlpb1"][0:128, :])
        mlpb1b = const.tile([P, 1], F32)
        nc.sync.dma_start(mlpb1b[:], ap["mlpb1"][128:256, :])
        mlpb2 = load("mlpb2", [128, 1])
        outw = load("outw", [P, 1], F16)
        outb = load("outb", [1, 1])
        maskb = load("maskb", [P, T])        # 0 / -1e9 additive mask, [b, t]

        gidx_sb = const.tile([P, T], I32)
        nc.sync.dma_start(gidx_sb[:], ap["gidx"][:])
        tidx_sb = const.tile([P, 1], I32)
        nc.sync.dma_start(tidx_sb[:], ap["tidx"][:])

        zeros = const.tile([P, P], F16)
        nc.vector.memset(zeros[:], 0.0)

        # big persistent buffers
        interest = const.tile([P, T * P], F16)   # [H, (t,b)] transposed
        haug = const.tile([P, P], F16)           # final AUGRU hidden [H, b]
        att_nat = const.tile([P, T], F16)        # attention weights [b, t]
        itemT = const.tile([P, P], F16)          # target item emb [D, b]
        q8 = const.tile([P, 8 * P], F16)         # itemT replicated 8x in free
        # gathered history embeddings, batch-native [b, (chunk, d)]
        histN = const.tile([P, T * P], F16)

        def mm(out, lhsT, rhs, start, stop):
            nc.tensor.matmul(out, lhsT, rhs, start=start, stop=stop)

        def it_slice(t, c):
            return interest[:, t * P + c * HB: t * P + (c + 1) * HB]

        def scan_step(pbank, gates, wih, whh, x_rhs, hprev, bc, out_slices):
            """One timestep for both 64-row chains.

            x_rhs(c) / hprev(c) / out_slices(c): APs for chain c.
            bc: [128,128] a_t broadcast tile, or None => plain GRU update.
            """
            banks = []
            for c in (0, 1):
                bank = pbank.tile([P, 256], F32, tag=f"bank{c}")
                r_, z_ = bank[:, 0:64], bank[:, 64:128]
                xn_, hn_ = bank[:, 128:192], bank[:, 192:256]
                xr = x_rhs(c)
                mm(r_, wih[:, 0:H], xr, start=True, stop=False)
                mm(z_, wih[:, H:2 * H], xr, start=False, stop=False)
                mm(xn_, wih[:, 2 * H:3 * H], xr, start=False, stop=False)
                hp = hprev(c)
                mm(hn_, whh[:, 2 * H:3 * H], hp, start=False, stop=False)
                mm(r_, whh[:, 0:H], hp, start=False, stop=False)
                mm(z_, whh[:, H:2 * H], hp, start=False, stop=True)
                banks.append(bank)
            rzs, ns, ws = [], [], []
            for c in (0, 1):
                rz = gates.tile([P, P], F16, tag=f"rz{c}")
                nc.scalar.activation(rz[:], banks[c][:, 0:128], AF.Sigmoid)
                rzs.append(rz)
                u = gates.tile([P, HB], F16, tag=f"u{c}")
                nc.vector.tensor_mul(u[:], rz[:, 0:64], banks[c][:, 192:256])
                v = gates.tile([P, HB], F16, tag=f"v{c}")
                nc.vector.tensor_add(v[:], u[:], banks[c][:, 128:192])
                n = gates.tile([P, HB], F16, tag=f"n{c}")
                nc.scalar.activation(n[:], v[:], AF.Tanh)
                ns.append(n)
                w = gates.tile([P, HB], F16, tag=f"w{c}")
                nc.gpsimd.tensor_sub(w[:], hprev(c), n[:])
                ws.append(w)
            for c in (0, 1):
                outsl = out_slices(c)
                if bc is None:
                    # GRU: h' = n + z*(h-n)
                    y = gates.tile([P, HB], F16, tag=f"y{c}")
                    nc.gpsimd.tensor_mul(y[:], rzs[c][:, 64:128], ws[c][:])
                    nc.vector.tensor_add(outsl, ns[c][:], y[:])
                else:
                    # AUGRU: h' = h - a*z*(h-n)
                    q = gates.tile([P, HB], F16, tag=f"q{c}")
                    nc.vector.tensor_mul(q[:], rzs[c][:, 64:128],
                                         bc[:, c * HB:(c + 1) * HB])
                    y = gates.tile([P, HB], F16, tag=f"y{c}")
                    nc.gpsimd.tensor_mul(y[:], q[:], ws[c][:])
                    nc.vector.tensor_sub(outsl, hprev(c), y[:])

        # =========================================================
        # Phase 0+1: item embedding + batched history gathers, then
        # GRU over T steps
        # =========================================================
        with tc.tile_pool(name="gnat", bufs=2) as gnat, \
             tc.tile_pool(name="xpos", bufs=4) as xpos, \
             tc.tile_pool(name="pbank", bufs=2, space="PSUM") as pbank, \
             tc.tile_pool(name="ptr", bufs=2, space="PSUM") as ptr, \
             tc.tile_pool(name="gates", bufs=3) as gates:

            item_nat = gnat.tile([P, P], F16, tag="item", name="item")
            nc.gpsimd.indirect_dma_start(
                out=item_nat[:], out_offset=None, in_=ap["emb"][:],
                in_offset=IndirectOffsetOnAxis(ap=tidx_sb[:, 0:1], axis=0))
            it_ps = ptr.tile([P, P], F16, tag="tr")
            nc.tensor.transpose(it_ps[:], item_nat[:], ident[:])
            nc.scalar.copy(itemT[:], it_ps[:])
            nc.vector.tensor_copy(q8[:], itemT[:].unsqueeze(1).to_broadcast([P, 8, P]))

            # batched history gathers: GCHUNK timesteps per indirect call
            for g0 in range(0, T, GCHUNK):
                gw = min(GCHUNK, T - g0)
                nc.gpsimd.indirect_dma_start(
                    out=histN[:, g0 * P:(g0 + gw) * P],
                    out_offset=None, in_=ap["emb"][:],
                    in_offset=IndirectOffsetOnAxis(
                        ap=gidx_sb[:, g0:g0 + gw], axis=0))

            for t in range(T):
                tp = ptr.tile([P, P], F16, tag="tr")
                nc.tensor.transpose(tp[:], histN[:, t * P:(t + 1) * P],
                                    ident[:])
                xT = xpos.tile([P, P], F16, tag="xT")
                if t % 2 == 0:
                    nc.scalar.copy(xT[:], tp[:])
                else:
                    nc.vector.tensor_copy(xT[:], tp[:])
                scan_step(
                    pbank, gates, g_wih, g_whh,
                    x_rhs=lambda c: xT[:, c * HB:(c + 1) * HB],
                    hprev=(lambda c: zeros[:, c * HB:(c + 1) * HB])
                    if t == 0 else (lambda c: it_slice(t - 1, c)),
                    bc=None,
                    out_slices=lambda c: it_slice(t, c))

        # =========================================================
        # Phase 2: attention scores + softmax + transpose to DRAM
        # =========================================================
        NT = T * P
        GRP = 1024
        with tc.tile_pool(name="apsum", bufs=2, space="PSUM") as apsum, \
             tc.tile_pool(name="ap2", bufs=1, space="PSUM") as ap2, \
             tc.tile_pool(name="asc", bufs=1, space="PSUM") as asc, \
             tc.tile_pool(name="asb", bufs=3) as asb:
            scores_ps = asc.tile([P, 2 * T], F32, tag="scores")
            for gi in range((NT + GRP - 1) // GRP):
                c0 = gi * GRP
                cw = min(GRP, NT - c0)
                iq = asb.tile([P, GRP], F16, tag="iq")
                eng = nc.vector if gi % 2 == 0 else nc.gpsimd
                eng.tensor_mul(iq[:, 0:cw], interest[:, c0:c0 + cw],
                               q8[:, 0:cw])
                ps1 = apsum.tile([64, GRP], F32, tag="ps1")
                for h2 in range(0, cw, 512):
                    hw = min(512, cw - h2)
                    seg = ps1[:, h2:h2 + hw]
                    mm(seg, attw1[:, 0:64],
                       interest[:, c0 + h2:c0 + h2 + hw],
                       start=True, stop=False)
                    mm(seg, attw1[:, 64:128], q8[:, 0:hw], start=False, stop=False)
                    mm(seg, attw1[:, 128:192], iq[:, h2:h2 + hw], start=False, stop=True)
                r1 = asb.tile([64, GRP], F16, tag="r1")
                nc.scalar.activation(r1[:, 0:cw], ps1[:, 0:cw], AF.Relu,
                                     bias=attb1[:])
                ps2 = ap2.tile([32, GRP], F32, tag="ps2")
                for h2 in range(0, cw, 512):
                    hw = min(512, cw - h2)
                    mm(ps2[:, h2:h2 + hw], attw2[:], r1[:, h2:h2 + hw], start=True, stop=True)
                r2 = asb.tile([32, GRP], F16, tag="r2")
                nc.scalar.activation(r2[:, 0:cw], ps2[:, 0:cw], AF.Relu,
                                     bias=attb2[:])
                for j in range(cw // P):
                    t = (c0 + j * P) // P
                    mm(scores_ps[:, 2 * t:2 * t + 2],
                       r2[:, j * P:(j + 1) * P], attwo2[:],
                       start=True, stop=True)

            sm = asb.tile([P, T], F32, tag="sm")
            nc.vector.tensor_add(sm[:], scores_ps[:, 0:2 * T:2], maskb[:])
            mx = asb.tile([P, 1], F32, tag="mx")
            nc.vector.tensor_reduce(mx[:], sm[:], axis=AX.X, op=OP.max)
            nmx = asb.tile([P, 1], F32, tag="nmx")
            nc.vector.tensor_scalar_mul(nmx[:], mx[:], -1.0)
            ex = asb.tile([P, T], F32, tag="ex")
            nc.scalar.activation(ex[:], sm[:], AF.Exp, bias=nmx[:])
            sume = asb.tile([P, 1], F32, tag="sume")
            nc.vector.tensor_reduce(sume[:], ex[:], axis=AX.X, op=OP.add)
            rec = asb.tile([P, 1], F32, tag="rec")
            nc.vector.reciprocal(rec[:], sume[:])
            nc.vector.tensor_scalar_mul(att_nat[:], ex[:], rec[:])

        with tc.tile_pool(name="atr", bufs=2, space="PSUM") as atr, \
             tc.tile_pool(name="atsb", bufs=2) as atsb:
            for k in range((T + P - 1) // P):
                tw = min(P, T - k * P)
                atp = atr.tile([P, P], F16, tag="tr")
                nc.tensor.transpose(atp[0:tw, :],
                                    att_nat[:, k * P:k * P + tw], ident[:])
                ats = atsb.tile([P, P], F16, tag="ats")
                nc.scalar.copy(ats[0:tw, :], atp[0:tw, :])
                nc.sync.dma_start(ap["attTd"][k * P:k * P + tw, :],
                                  ats[0:tw, :])

        # =========================================================
        # Phase 3: AUGRU
        # =========================================================
        with tc.tile_pool(name="pbank2", bufs=2, space="PSUM") as pbank, \
             tc.tile_pool(name="gates2", bufs=3) as gates, \
             tc.tile_pool(name="hpool", bufs=4) as hpool, \
             tc.tile_pool(name="bcp", bufs=4) as bcp:
            ah_prev = [None, None]
            for t in range(T):
                bc = bcp.tile([P, P], F16, tag="bc")
                nc.sync.dma_start(
                    bc[:], ap["attTd"][t:t + 1, :].to_broadcast([P, P]))
                ah_new = [hpool.tile([P, HB], F16, tag=f"ah{c}",
                                     name=f"ah{c}")
                          for c in (0, 1)] if t < T - 1 else None

                def out_sl(c, t=t, ah_new=ah_new):
                    if t == T - 1:
                        return haug[:, c * HB:(c + 1) * HB]
                    return ah_new[c][:]

                scan_step(
                    pbank, gates, a_wih, a_whh,
                    x_rhs=lambda c, t=t: it_slice(t, c),
                    hprev=(lambda c: zeros[:, c * HB:(c + 1) * HB]) if t == 0
                    else (lambda c, hp=ah_prev: hp[c][:]),
                    bc=bc,
                    out_slices=out_sl)
                ah_prev = ah_new

        # =========================================================
        # Phase 4: final MLP
        # =========================================================
        with tc.tile_pool(name="mpsum", bufs=1, space="PSUM") as mpsum, \
             tc.tile_pool(name="msb", bufs=1) as msb:
            hdn = []
            for m in range(2):
                ps = mpsum.tile([P, P], F32, tag=f"mlp1_{m}")
                mm(ps[:], mlpw1a[:, m * 128:(m + 1) * 128], haug[:], start=True, stop=False)
                mm(ps[:], mlpw1b[:, m * 128:(m + 1) * 128], itemT[:], start=False, stop=True)
                hd = msb.tile([P, P], F16, tag=f"hdn{m}")
                nc.scalar.activation(hd[:], ps[:], AF.Relu,
                                     bias=(mlpb1a if m == 0 else mlpb1b)[:])
                hdn.append(hd)
            ps = mpsum.tile([P, P], F32, tag="mlp2")
            mm(ps[:], mlpw2a[:], hdn[0][:], start=True, stop=False)
            mm(ps[:], mlpw2b[:], hdn[1][:], start=False, stop=True)
            hd2 = msb.tile([P, P], F16, tag="hdn2")
            nc.scalar.activation(hd2[:], ps[:], AF.Relu, bias=mlpb2[:])
            pso = mpsum.tile([1, P], F32, tag="mlpo")
            mm(pso[:], outw[:], hd2[:], start=True, stop=True)
            res = msb.tile([1, P], F32, tag="res")
            nc.scalar.activation(res[:], pso[:], AF.Identity, bias=outb[:])
            nc.sync.dma_start(ap["out"][:], res[:])


def _build(T):
    nc = bacc.Bacc("TRN2", target_bir_lowering=False, debug=False)
    ap = {}

    def din(name, shape, dt=F32):
        ap[name] = nc.dram_tensor(name, shape, dt, kind="ExternalInput").ap()

    din("emb", [V, D], F16)
    din("gidx", [BS, T], I32)
    din("tidx", [BS, 1], I32)
    din("maskb", [BS, T])
    din("g_wihT", [D, 3 * H], F16); din("g_whhT", [H, 3 * H], F16)
    din("a_wihT", [H, 3 * H], F16); din("a_whhT", [H, 3 * H], F16)
    din("attw1", [D, 192], F16); din("attw2", [64, 32], F16); din("attwo2", [32, 2], F16)
    din("attb1", [64, 1]); din("attb2", [32, 1])
    din("mlpw1", [256, 256], F16); din("mlpb1", [256, 1])
    din("mlpw2", [256, 128], F16); din("mlpb2", [128, 1])
    din("outw", [128, 1], F16); din("outb", [1, 1])
    ap["attTd"] = nc.dram_tensor("attTd", [((T + P - 1) // P) * P, P], F16).ap()
    ap["out"] = nc.dram_tensor("out", [1, BS], F32, kind="ExternalOutput").ap()
    _emit(nc, T, ap)
    nc.compile()
    return nc


def _prep_inputs(inputs, T):
    """Host-side sharding / weight layout prep. Returns list of 8 dicts."""
    f16 = lambda x: np.ascontiguousarray(np.asarray(x, dtype=np.float16))
    f = lambda x: np.ascontiguousarray(np.asarray(x, dtype=np.float32))
    emb = f16(inputs["emb"])
    hist = np.asarray(inputs["history_items"]).astype(np.int32)
    tgt = np.asarray(inputs["target_item"]).astype(np.int32)
    mask = np.asarray(inputs["history_mask"])
    maskb = np.where(mask, 0.0, -1e9).astype(np.float32)

    def gate_T(w):  # [3H, K] -> [K, 3H] with r|z|n column blocks
        return np.ascontiguousarray(np.asarray(w, dtype=np.float32).T.astype(np.float16))

    for nm in ("gru_bih", "gru_bhh", "augru_bih", "augru_bhh"):
        assert np.abs(np.asarray(inputs[nm])).max() == 0.0, \
            f"nonzero bias {nm} unsupported"
    W1 = f(inputs["att_W1"])  # [4H, 64]
    Ai = W1[0:H] + W1[2 * H:3 * H]
    Aq = W1[H:2 * H] - W1[2 * H:3 * H]
    Apr = W1[3 * H:4 * H]
    attw1 = np.ascontiguousarray(
        np.concatenate([Ai, Aq, Apr], axis=1).astype(np.float16))
    common = dict(
        emb=emb,
        g_wihT=gate_T(inputs["gru_Wih"]), g_whhT=gate_T(inputs["gru_Whh"]),
        a_wihT=gate_T(inputs["augru_Wih"]), a_whhT=gate_T(inputs["augru_Whh"]),
        attw1=attw1, attw2=f16(inputs["att_W2"]),
        attwo2=np.ascontiguousarray(
            np.repeat(f(inputs["att_Wo"]), 2, axis=1).astype(np.float16)),
        attb1=f(inputs["att_b1"]).reshape(64, 1),
        attb2=f(inputs["att_b2"]).reshape(32, 1),
        mlpw1=f16(inputs["mlp_W1"]), mlpb1=f(inputs["mlp_b1"]).reshape(256, 1),
        mlpw2=f16(inputs["mlp_W2"]), mlpb2=f(inputs["mlp_b2"]).reshape(128, 1),
        outw=f16(inputs["out_W"]), outb=f(inputs["out_b"]).reshape(1, 1),
    )
    ins = []
    for c in range(NCORES):
        s = slice(c * BS, (c + 1) * BS)
        m = dict(common)
        m["gidx"] = np.ascontiguousarray(hist[s, :T])
        m["tidx"] = np.ascontiguousarray(tgt[s].reshape(BS, 1))
        m["maskb"] = np.ascontiguousarray(maskb[s, :T])
        ins.append(m)
    return ins


_CACHE = {}


def run(inputs, T):
    if T not in _CACHE:
        _CACHE[T] = _build(T)
    nc = _CACHE[T]
    ins = _prep_inputs(inputs, T)
    res = run_bass_kernel_spmd(nc, ins, core_ids=list(range(NCORES)))
    out = np.concatenate([res.results[c]["out"].reshape(BS)
                          for c in range(NCORES)])
    return out.reshape(B, 1).astype(np.float32)


def kernel(**inputs):
    T = np.asarray(inputs["history_items"]).shape[1]
    return run(inputs, T)
